# revision 40
# baseline (speedup 1.0000x reference)
"""Multi-head attention (B=4, N=2048, C=1024, H=16, D=64) on 8 TRN2 NeuronCores.

Sharding: core c handles batch b = c//2 and head-group g = c%2 (8 heads = 512
dims).  Each core computes qkv projection, attention, and a partial output
projection for its head slice; the host sums the two partials per batch and
adds the proj bias.

Per-core kernel (bf16 data, fp32 PSUM accumulation everywhere):
  host prep: x is pre-transposed to [C, N] bf16 per batch (removes all PE
  transposes); W_qkv / W_proj pre-cast to bf16 (halves weight DMA + PE
  weight-load time; bf16 loads hide fully under the 512-row streams).
  phase 1: DMA xT slabs; qT/kT (d-major) and augmented v (n-major, per-head
           65th column of ones) via accumulating matmuls.
  phase 2: per (slab, head-pair): S^T = k @ q^T in PSUM (row-packed pairs)
           -> one exp per chunk on ScalarE (scale=1/8 folded in; no max
           subtraction, logits ~ N(0,1)) -> bf16 e tiles -> PV against
           augmented v gives P@V rows 0..63 + softmax denominator row 64 in
           one accumulation group.  PV trails exp by >=2 chunks and is
           emitted sub-major in 4-chunk blocks so consecutive matmuls
           accumulate into the same PSUM bank (a bank switch costs ~170ns).
           Normalization: one reciprocal over both heads' stacked
           denominators (DVE, emitted at pair end), then the PE-side
           broadcast matmuls + multiply are DEFERRED into the next pair's
           chunk loop so the PE never stalls on the reciprocal chain.
           q^T for slab s+1 is computed inside slab s's pairs (one
           accumulating matmul per chunk over the first 8 chunks) -- the
           attention phase is exp-paced and absorbs it.  The slab's output
           projection (double-buffered PSUM) runs when its last pair
           completes.
  Dummy matmuls at kernel start / phase boundary / tail keep the PE HAM
  clock gate at 2.4 GHz (idle >3.4us re-throttles to 1.2 GHz).
"""

from contextlib import ExitStack

import ml_dtypes
import numpy as np

import concourse.bass as bass
import concourse.tile as tile
from concourse import bacc, mybir
from concourse.bass_utils import run_bass_kernel_spmd

P = 128
N = 2048          # tokens per batch
C = 1024          # model dim
DC = 512          # head dims per core (8 heads x 64)
NSLABS = N // 512
F32 = mybir.dt.float32
BF16 = mybir.dt.bfloat16


def build_program(trace_label: str = "attn"):
    nc = bacc.Bacc("TRN2", target_bir_lowering=False, name=trace_label)
    # x arrives pre-transposed (host-side): [C, N] bf16, so no PE transposes.
    xt_d = nc.dram_tensor("xt", [C, N], BF16, kind="ExternalInput").ap()
    wqkv_d = nc.dram_tensor("wqkv", [C, 3 * DC], BF16, kind="ExternalInput").ap()
    wproj_d = nc.dram_tensor("wproj", [DC, C], BF16, kind="ExternalInput").ap()
    out_d = nc.dram_tensor("out", [N, C], F32, kind="ExternalOutput").ap()

    with tile.TileContext(nc) as tc, ExitStack() as ctx:
        _emit(ctx, tc, xt_d, wqkv_d, wproj_d, out_d)
    nc.compile()
    return nc


def _emit(ctx, tc, xt_d, wqkv_d, wproj_d, out_d):
    nc = tc.nc

    const = ctx.enter_context(tc.tile_pool(name="const", bufs=1))
    # warm-up operand: contents are irrelevant (the HAM clock gate only sees
    # PE activity); a memset is ready in ~100ns vs ~9us for a gpsimd-built
    # identity, so the PE warms while the first DMAs are still in flight
    ident = const.tile([P, P], BF16, tag="ident")
    nc.any.memset(ident.bitcast(mybir.dt.uint32), 0)
    ONE_BF16_BITS = 0x3F803F80  # two packed bf16 1.0s; memset writes raw bits
    ones_row = const.tile([1, 64], BF16, tag="ones_row")  # lhsT for broadcast
    nc.any.memset(ones_row.bitcast(mybir.dt.uint32), ONE_BF16_BITS)

    # Persistent SBUF tensors (d-major q/k in bf16, n-major v in fp8).
    # v is stored augmented: per head 65 columns, the 65th = 1.0, so a single
    # accumulating matmul yields both P@V (rows 0..63) and the softmax
    # denominator (row 64) without a second accumulation group in the bank.
    persist = ctx.enter_context(tc.tile_pool(name="persist", bufs=1))
    qT = persist.tile([P, 4, N], BF16, tag="qT")          # [d%128, d//128, n]
    kT = persist.tile([P, 4, N], BF16, tag="kT")
    va = persist.tile([P, N // P, 8 * 65], BF16, tag="va")  # [n%128, n//128, 65*h+dd]
    nc.any.memset(va[:].bitcast(mybir.dt.uint32), ONE_BF16_BITS)

    # ---------------- phase 1: k/v projection (+ q for slab 0) ----------------
    # wq and the xT tiles stay resident: qT for slab s+1 is computed inside
    # phase 2's slab s (the attention phase is exp-paced, so the PE has slack).
    wpool = ctx.enter_context(tc.tile_pool(name="wqkv", bufs=1))
    xt_pool = ctx.enter_context(tc.tile_pool(name="xt", bufs=4))
    xts = []
    with tc.tile_pool(name="ps_warm", bufs=1, space="PSUM") as ps_warm, \
         tc.tile_pool(name="ps_qkv", bufs=4, space="PSUM") as ps_qkv:

        # PE warm-up: the HAM clock gate needs ~3.4us of sustained matmul
        # activity to lift the PE from 1.2 to 2.4 GHz. Spin harmless matmuls
        # while the first x/W DMAs are in flight so real work starts warm.
        warm = ps_warm.tile([P, P], F32, tag="warm")
        for _ in range(48):
            nc.tensor.matmul(warm[:], ident[:], ident[:])

        # all xT slab DMAs are issued upfront; W rides the scalar-engine DGE
        # queue so it streams in parallel.
        wq = wpool.tile([P, 8, 3 * DC], BF16, tag="wqkv")  # [c%128, c//128, col]
        for s in range(NSLABS):
            xt = xt_pool.tile([P, 8, 512], BF16, tag="xt",
                              name=f"xt{s}")  # [c%128, c//128, n]
            for cc in range(8):
                nc.sync.dma_start(
                    xt[:, cc, :],
                    xt_d[cc * P:(cc + 1) * P, s * 512:(s + 1) * 512],
                )
            xts.append(xt)
            if s == 0:
                for cc in range(8):
                    nc.scalar.dma_start(wq[:, cc, :], wqkv_d[cc * P:(cc + 1) * P, :])

        for s in range(NSLABS):
            xt = xts[s]
            # k^T for every slab; q^T only for slab 0 (rest come in phase 2)
            for dst, base in ((kT, DC), (qT, 0)) if s == 0 else ((kT, DC),):
                for dc in range(4):
                    ps = ps_qkv.tile([P, 512], F32, tag="qkv")
                    col = base + dc * P
                    for cc in range(8):
                        nc.tensor.matmul(
                            ps[:],
                            wq[:, cc, col:col + P],
                            xt[:, cc, :],
                            start=(cc == 0),
                            stop=(cc == 7),
                        )
                    nc.vector.tensor_copy(dst[:, dc, s * 512:(s + 1) * 512], ps[:])
            # v (natural layout, scattered into the 65-wide augmented blocks)
            for i in range(4):
                ps = ps_qkv.tile([P, 512], F32, tag="qkv")
                for cc in range(8):
                    nc.tensor.matmul(
                        ps[:],
                        xt[:, cc, i * P:(i + 1) * P],
                        wq[:, cc, 2 * DC:3 * DC],
                        start=(cc == 0),
                        stop=(cc == 7),
                    )
                for h in range(8):
                    nc.vector.tensor_copy(
                        va[:, 4 * s + i, 65 * h:65 * h + 64],
                        ps[:, 64 * h:64 * h + 64],
                    )

        # keep the PE busy across the phase boundary (PSUM-bank WAR waits
        # would otherwise idle it past the HAM re-throttle window)
        for _ in range(16):
            nc.tensor.matmul(warm[:], ident[:], ident[:])

    # ---------------- phase 2: attention ----------------
    # aT is only written from phase 2 on; allocating it here (after the
    # phase-1 pools release) keeps phase-1 SBUF under budget.
    attn_persist = ctx.enter_context(tc.tile_pool(name="attn_persist", bufs=1))
    aT = attn_persist.tile([P, 4, N], BF16, tag="aT")     # attn_out^T
    with tc.tile_pool(name="ps_st", bufs=2, space="PSUM") as ps_st, \
         tc.tile_pool(name="ps_pv", bufs=2, space="PSUM") as ps_pv, \
         tc.tile_pool(name="ps_proj", bufs=2, space="PSUM") as ps_proj, \
         tc.tile_pool(name="etile", bufs=8) as epool, \
         tc.tile_pool(name="norm", bufs=4) as npool, \
         tc.tile_pool(name="wproj", bufs=1) as wp_pool, \
         tc.tile_pool(name="oproj", bufs=2) as opool:

        wp = wp_pool.tile([P, 4, C], BF16, tag="wp")
        for dc in range(4):
            nc.scalar.dma_start(wp[:, dc, :], wproj_d[dc * P:(dc + 1) * P, :])

        # the previous pair's PV tail, PSUM drain, and normalization are
        # deferred into the next pair's first chunks (one piece per chunk) so
        # the next pair's scores/exp start immediately at the boundary and
        # the exp stream never waits on the drain chain
        deferred = []
        for s in range(NSLABS):          # 512-wide n_q slab (outer: spreads proj)
            for pair in range(4):        # heads (2*pair, 2*pair+1); d-chunk=pair
                pvs = [
                    ps_pv.tile([P, 512], F32, tag="pv", name=f"pv{pair}_{s}_{i}")
                    for i in range(2)
                ]
                def emit_pv(batch, pvs=pvs, pair=pair):
                    # sub-major over a block of chunks: consecutive matmuls
                    # accumulate into the SAME PSUM bank (alternating banks
                    # per matmul costs ~170ns each on HW)
                    for sub in range(2):
                        h = 2 * pair + sub
                        for e_prev, ck_prev in batch:
                            nc.tensor.matmul(
                                pvs[sub][0:65, :],
                                va[:, ck_prev, 65 * h:65 * h + 65],
                                e_prev[:, sub, :],
                                start=(ck_prev == 0),
                                stop=(ck_prev == N // P - 1),
                            )

                # q^T d-chunk `pair` of the NEXT slab rides this pair's
                # exp-paced slack: one accumulating matmul per chunk, spread
                # over the first 8 chunks (the PV pipeline is still filling
                # there, so single-matmul insertions don't starve the exp)
                qps = None
                if s + 1 < NSLABS:
                    qps = ps_proj.tile([P, 512], F32, tag="proj",
                                       name=f"q_{s + 1}_{pair}")

                # software pipeline: PV trails exp by >=2 chunks (so the PE's
                # PV waits are pre-satisfied) and is emitted in blocks of 4
                # chunks for the same-bank runs above
                pending = []
                for ck in range(N // P):  # 128-wide n_k chunk
                    st = ps_st.tile([P, 2, 512], F32, tag="st")
                    for sub in range(2):
                        o = 64 * sub
                        nc.tensor.matmul(
                            st[:, sub, :],
                            kT[o:o + 64, pair, ck * P:(ck + 1) * P],
                            qT[o:o + 64, pair, s * 512:(s + 1) * 512],
                        )
                    e = epool.tile([P, 2, 512], BF16, tag="e")
                    nc.scalar.activation(
                        e[:], st[:], mybir.ActivationFunctionType.Exp, scale=0.125
                    )
                    if ck < len(deferred):
                        deferred[ck]()
                        if ck == len(deferred) - 1:
                            deferred = []
                    if qps is not None and ck < 8:
                        nc.tensor.matmul(
                            qps[:],
                            wq[:, ck, pair * P:(pair + 1) * P],
                            xts[s + 1][:, ck, :],
                            start=(ck == 0),
                            stop=(ck == 7),
                        )
                        if ck == 7:
                            nc.vector.tensor_copy(
                                qT[:, pair, (s + 1) * 512:(s + 2) * 512], qps[:])
                    pending.append((e, ck))
                    if len(pending) == 6:
                        emit_pv(pending[:4])
                        pending = pending[4:]
                tail0, tail1 = pending[:2], pending[2:]

                # drain chain pieces: pv copies + denominator reciprocal
                # (DVE), then the PE-side broadcast + multiply.  cell carries
                # rc from the first piece to the second.
                cell = {}

                def copies_rc(pvs=pvs, pair=pair, s=s, cell=cell):
                    # aT[64*sub.., pair, slab] = pv[0:64]; dn row = pv[64]
                    for sub in range(2):
                        nc.vector.tensor_copy(
                            aT[64 * sub:64 * sub + 64, pair,
                               s * 512:(s + 1) * 512],
                            pvs[sub][0:64, :],
                        )
                    dn = npool.tile([1, 2, 512], F32, tag="dn",
                                    name=f"dn_{pair}_{s}")
                    for sub in range(2):
                        nc.vector.tensor_copy(dn[:, sub, :], pvs[sub][64:65, :])
                    rc32 = npool.tile([1, 2, 512], F32, tag="rc32",
                                      name=f"rc32_{pair}_{s}")
                    nc.vector.reciprocal_approx_fast(rc32[:], dn[:])
                    rc = npool.tile([1, 2, 512], BF16, tag="rc",
                                    name=f"rc_{pair}_{s}")
                    nc.vector.tensor_copy(rc[:], rc32[:])
                    cell["rc"] = rc

                def finish_norm(pair=pair, s=s, cell=cell):
                    rc = cell["rc"]
                    bcs = npool.tile([P, 512], BF16, tag="bcs",
                                     name=f"bcs_{pair}_{s}")
                    for sub in range(2):
                        bc = ps_proj.tile([P, 512], F32, tag="proj",
                                          name=f"bc_{pair}_{s}_{sub}")
                        nc.tensor.matmul(bc[0:64, :], ones_row[:], rc[:, sub, :])
                        o = 64 * sub
                        nc.vector.tensor_copy(bcs[o:o + 64, :], bc[0:64, :])
                    sl = aT[:, pair, s * 512:(s + 1) * 512]
                    nc.vector.tensor_mul(sl, sl, bcs[:])

                def make_proj(nck, ct):
                    def f():
                        pp = ps_proj.tile([P, 512], F32, tag="proj",
                                          name=f"proj{nck}_{ct}")
                        for dc in range(4):
                            nc.tensor.matmul(
                                pp[:],
                                aT[:, dc, nck * P:(nck + 1) * P],
                                wp[:, dc, ct * 512:(ct + 1) * 512],
                                start=(dc == 0),
                                stop=(dc == 3),
                            )
                        ot = opool.tile([P, 512], F32, tag="ot")
                        nc.vector.tensor_copy(ot[:], pp[:])
                        nc.sync.dma_start(
                            out_d[nck * P:(nck + 1) * P,
                                  ct * 512:(ct + 1) * 512],
                            ot[:],
                        )
                    return f

                drain = [
                    lambda b=tail0, f=emit_pv: f(b),
                    lambda b=tail1, f=emit_pv: f(b),
                    copies_rc,
                    finish_norm,
                ]
                if pair == 3:
                    # this slab's projection follows its normalization; both
                    # are deferred into the next slab's first chunks (or run
                    # inline for the final slab)
                    projs = [make_proj(4 * s + i, ct)
                             for i in range(4) for ct in range(2)]
                    if s == NSLABS - 1:
                        for f in drain[:2]:
                            f()
                        pwarm = ps_proj.tile([P, 512], F32, tag="proj",
                                             name="proj_warm")
                        for _ in range(32):
                            nc.tensor.matmul(pwarm[:, 0:P], ident[:], ident[:])
                        for f in drain[2:] + projs:
                            f()
                    else:
                        deferred = drain + projs
                else:
                    deferred = drain



def shard_inputs(x, W_qkv, W_proj):
    """Full inputs -> 8 per-core in_maps. Core c: batch c//2, head-group c%2."""
    x = np.asarray(x, dtype=np.float32)
    W_qkv = np.asarray(W_qkv, dtype=np.float32)
    W_proj = np.asarray(W_proj, dtype=np.float32)
    in_maps = []
    for core in range(8):
        b, g = core // 2, core % 2
        cols = slice(g * DC, (g + 1) * DC)
        w = np.concatenate(
            [W_qkv[:, 0:C][:, cols], W_qkv[:, C:2 * C][:, cols],
             W_qkv[:, 2 * C:3 * C][:, cols]],
            axis=1,
        )
        in_maps.append({
            "xt": np.ascontiguousarray(x[b].T).astype(ml_dtypes.bfloat16),
            "wqkv": np.ascontiguousarray(w).astype(ml_dtypes.bfloat16),
            "wproj": np.ascontiguousarray(
                W_proj[g * DC:(g + 1) * DC, :]).astype(ml_dtypes.bfloat16),
        })
    return in_maps


def unshard_output(results, b_proj):
    b_proj = np.asarray(b_proj, dtype=np.float32)
    out = np.empty((4, N, C), dtype=np.float32)
    for b in range(4):
        out[b] = results[2 * b]["out"] + results[2 * b + 1]["out"] + b_proj[None, :]
    return out


_NC_CACHE = []


def kernel(x, W_qkv, W_proj, b_proj, trace=False):
    in_maps = shard_inputs(x, W_qkv, W_proj)
    if not _NC_CACHE:
        _NC_CACHE.append(build_program())
    nc = _NC_CACHE[0]
    res = run_bass_kernel_spmd(nc, in_maps, core_ids=list(range(8)), trace=trace)
    out = unshard_output(res.results, b_proj)
    if trace:
        return out, res
    return out


# revision 41
# speedup vs baseline: 1.0045x; 1.0045x over previous
"""Multi-head attention (B=4, N=2048, C=1024, H=16, D=64) on 8 TRN2 NeuronCores.

Sharding: core c handles batch b = c//2 and head-group g = c%2 (8 heads = 512
dims).  Each core computes qkv projection, attention, and a partial output
projection for its head slice; the host sums the two partials per batch and
adds the proj bias.

Per-core kernel (bf16 data, fp32 PSUM accumulation everywhere):
  host prep: x is pre-transposed to [C, N] bf16 per batch (removes all PE
  transposes); W_qkv / W_proj pre-cast to bf16 (halves weight DMA + PE
  weight-load time; bf16 loads hide fully under the 512-row streams).
  phase 1: DMA xT slabs; qT/kT (d-major) and augmented v (n-major, per-head
           65th column of ones) via accumulating matmuls.
  phase 2: per (slab, head-pair): S^T = k @ q^T in PSUM (row-packed pairs)
           -> one exp per chunk on ScalarE (scale=1/8 folded in; no max
           subtraction, logits ~ N(0,1)) -> bf16 e tiles -> PV against
           augmented v gives P@V rows 0..63 + softmax denominator row 64 in
           one accumulation group.  PV trails exp by >=2 chunks and is
           emitted sub-major in 4-chunk blocks so consecutive matmuls
           accumulate into the same PSUM bank (a bank switch costs ~170ns).
           Normalization: one reciprocal over both heads' stacked
           denominators (DVE, emitted at pair end), then the PE-side
           broadcast matmuls + multiply are DEFERRED into the next pair's
           chunk loop so the PE never stalls on the reciprocal chain.
           q^T for slab s+1 is computed inside slab s's pairs (one
           accumulating matmul per chunk over the first 8 chunks) -- the
           attention phase is exp-paced and absorbs it.  The slab's output
           projection (double-buffered PSUM) runs when its last pair
           completes.
  Dummy matmuls at kernel start / phase boundary / tail keep the PE HAM
  clock gate at 2.4 GHz (idle >3.4us re-throttles to 1.2 GHz).
"""

from contextlib import ExitStack

import ml_dtypes
import numpy as np

import concourse.bass as bass
import concourse.tile as tile
from concourse import bacc, mybir
from concourse.bass_utils import run_bass_kernel_spmd

P = 128
N = 2048          # tokens per batch
C = 1024          # model dim
DC = 512          # head dims per core (8 heads x 64)
NSLABS = N // 512
F32 = mybir.dt.float32
BF16 = mybir.dt.bfloat16


def build_program(trace_label: str = "attn"):
    nc = bacc.Bacc("TRN2", target_bir_lowering=False, name=trace_label)
    # x arrives pre-transposed (host-side): [C, N] bf16, so no PE transposes.
    xt_d = nc.dram_tensor("xt", [C, N], BF16, kind="ExternalInput").ap()
    wqkv_d = nc.dram_tensor("wqkv", [C, 3 * DC], BF16, kind="ExternalInput").ap()
    wproj_d = nc.dram_tensor("wproj", [DC, C], BF16, kind="ExternalInput").ap()
    out_d = nc.dram_tensor("out", [N, C], F32, kind="ExternalOutput").ap()

    with tile.TileContext(nc) as tc, ExitStack() as ctx:
        _emit(ctx, tc, xt_d, wqkv_d, wproj_d, out_d)
    nc.compile()
    return nc


def _emit(ctx, tc, xt_d, wqkv_d, wproj_d, out_d):
    nc = tc.nc

    const = ctx.enter_context(tc.tile_pool(name="const", bufs=1))
    # warm-up operand: contents are irrelevant (the HAM clock gate only sees
    # PE activity); a memset is ready in ~100ns vs ~9us for a gpsimd-built
    # identity, so the PE warms while the first DMAs are still in flight
    ident = const.tile([P, P], BF16, tag="ident")
    nc.any.memset(ident.bitcast(mybir.dt.uint32), 0)
    ONE_BF16_BITS = 0x3F803F80  # two packed bf16 1.0s; memset writes raw bits
    ones_row = const.tile([1, 64], BF16, tag="ones_row")  # lhsT for broadcast
    nc.any.memset(ones_row.bitcast(mybir.dt.uint32), ONE_BF16_BITS)

    # Persistent SBUF tensors (d-major q/k in bf16, n-major v in fp8).
    # v is stored augmented: per head 65 columns, the 65th = 1.0, so a single
    # accumulating matmul yields both P@V (rows 0..63) and the softmax
    # denominator (row 64) without a second accumulation group in the bank.
    persist = ctx.enter_context(tc.tile_pool(name="persist", bufs=1))
    qT = persist.tile([P, 4, N], BF16, tag="qT")          # [d%128, d//128, n]
    kT = persist.tile([P, 4, N], BF16, tag="kT")
    va = persist.tile([P, N // P, 8 * 65], BF16, tag="va")  # [n%128, n//128, 65*h+dd]
    nc.any.memset(va[:].bitcast(mybir.dt.uint32), ONE_BF16_BITS)

    # ---------------- phase 1: k/v projection (+ q for slab 0) ----------------
    # wq and the xT tiles stay resident: qT for slab s+1 is computed inside
    # phase 2's slab s (the attention phase is exp-paced, so the PE has slack).
    wpool = ctx.enter_context(tc.tile_pool(name="wqkv", bufs=1))
    xt_pool = ctx.enter_context(tc.tile_pool(name="xt", bufs=4))
    xts = []
    with tc.tile_pool(name="ps_warm", bufs=1, space="PSUM") as ps_warm, \
         tc.tile_pool(name="ps_qkv", bufs=4, space="PSUM") as ps_qkv:

        # PE warm-up: the HAM clock gate needs ~3.4us of sustained matmul
        # activity to lift the PE from 1.2 to 2.4 GHz. Spin harmless matmuls
        # while the first x/W DMAs are in flight so real work starts warm.
        warm = ps_warm.tile([P, P], F32, tag="warm")
        for _ in range(48):
            nc.tensor.matmul(warm[:], ident[:], ident[:])

        # all xT slab DMAs are issued upfront; W rides the scalar-engine DGE
        # queue so it streams in parallel.
        wq = wpool.tile([P, 8, 3 * DC], BF16, tag="wqkv")  # [c%128, c//128, col]
        for s in range(NSLABS):
            xt = xt_pool.tile([P, 8, 512], BF16, tag="xt",
                              name=f"xt{s}")  # [c%128, c//128, n]
            for cc in range(8):
                nc.sync.dma_start(
                    xt[:, cc, :],
                    xt_d[cc * P:(cc + 1) * P, s * 512:(s + 1) * 512],
                )
            xts.append(xt)
            if s == 0:
                for cc in range(8):
                    nc.scalar.dma_start(wq[:, cc, :], wqkv_d[cc * P:(cc + 1) * P, :])

        for s in range(NSLABS):
            xt = xts[s]
            # k^T for every slab; q^T only for slab 0 (rest come in phase 2)
            for dst, base in ((kT, DC), (qT, 0)) if s == 0 else ((kT, DC),):
                for dc in range(4):
                    ps = ps_qkv.tile([P, 512], F32, tag="qkv")
                    col = base + dc * P
                    for cc in range(8):
                        nc.tensor.matmul(
                            ps[:],
                            wq[:, cc, col:col + P],
                            xt[:, cc, :],
                            start=(cc == 0),
                            stop=(cc == 7),
                        )
                    nc.vector.tensor_copy(dst[:, dc, s * 512:(s + 1) * 512], ps[:])
            # v (natural layout, scattered into the 65-wide augmented blocks)
            for i in range(4):
                ps = ps_qkv.tile([P, 512], F32, tag="qkv")
                for cc in range(8):
                    nc.tensor.matmul(
                        ps[:],
                        xt[:, cc, i * P:(i + 1) * P],
                        wq[:, cc, 2 * DC:3 * DC],
                        start=(cc == 0),
                        stop=(cc == 7),
                    )
                for h in range(8):
                    nc.vector.tensor_copy(
                        va[:, 4 * s + i, 65 * h:65 * h + 64],
                        ps[:, 64 * h:64 * h + 64],
                    )

        # keep the PE busy across the phase boundary (PSUM-bank WAR waits
        # would otherwise idle it past the HAM re-throttle window)
        for _ in range(16):
            nc.tensor.matmul(warm[:], ident[:], ident[:])

    # ---------------- phase 2: attention ----------------
    # aT is only written from phase 2 on; allocating it here (after the
    # phase-1 pools release) keeps phase-1 SBUF under budget.
    attn_persist = ctx.enter_context(tc.tile_pool(name="attn_persist", bufs=1))
    aT = attn_persist.tile([P, 4, N], BF16, tag="aT")     # attn_out^T
    with tc.tile_pool(name="ps_st", bufs=2, space="PSUM") as ps_st, \
         tc.tile_pool(name="ps_pv", bufs=2, space="PSUM") as ps_pv, \
         tc.tile_pool(name="ps_proj", bufs=2, space="PSUM") as ps_proj, \
         tc.tile_pool(name="etile", bufs=8) as epool, \
         tc.tile_pool(name="norm", bufs=4) as npool, \
         tc.tile_pool(name="wproj", bufs=1) as wp_pool, \
         tc.tile_pool(name="oproj", bufs=2) as opool:

        wp = wp_pool.tile([P, 4, C], BF16, tag="wp")
        for dc in range(4):
            nc.scalar.dma_start(wp[:, dc, :], wproj_d[dc * P:(dc + 1) * P, :])

        # the previous pair's PV tail, PSUM drain, and normalization are
        # deferred into the next pair's first chunks (one piece per chunk) so
        # the next pair's scores/exp start immediately at the boundary and
        # the exp stream never waits on the drain chain
        deferred = []
        for s in range(NSLABS):          # 512-wide n_q slab (outer: spreads proj)
            for pair in range(4):        # heads (2*pair, 2*pair+1); d-chunk=pair
                pvs = [
                    ps_pv.tile([P, 512], F32, tag="pv", name=f"pv{pair}_{s}_{i}")
                    for i in range(2)
                ]
                def emit_pv(batch, pvs=pvs, pair=pair):
                    # sub-major over a block of chunks: consecutive matmuls
                    # accumulate into the SAME PSUM bank (alternating banks
                    # per matmul costs ~170ns each on HW)
                    for sub in range(2):
                        h = 2 * pair + sub
                        for e_prev, ck_prev in batch:
                            nc.tensor.matmul(
                                pvs[sub][0:65, :],
                                va[:, ck_prev, 65 * h:65 * h + 65],
                                e_prev[:, sub, :],
                                start=(ck_prev == 0),
                                stop=(ck_prev == N // P - 1),
                            )

                # q^T d-chunk `pair` of the NEXT slab rides this pair's
                # exp-paced slack: one accumulating matmul per chunk, spread
                # over the first 8 chunks (the PV pipeline is still filling
                # there, so single-matmul insertions don't starve the exp)
                qps = None
                if s + 1 < NSLABS:
                    qps = ps_proj.tile([P, 512], F32, tag="proj",
                                       name=f"q_{s + 1}_{pair}")

                # software pipeline: PV trails exp by >=2 chunks (so the PE's
                # PV waits are pre-satisfied) and is emitted in blocks of 4
                # chunks for the same-bank runs above
                pending = []
                for ck in range(N // P):  # 128-wide n_k chunk
                    st = ps_st.tile([P, 2, 512], F32, tag="st")
                    for sub in range(2):
                        o = 64 * sub
                        nc.tensor.matmul(
                            st[:, sub, :],
                            kT[o:o + 64, pair, ck * P:(ck + 1) * P],
                            qT[o:o + 64, pair, s * 512:(s + 1) * 512],
                        )
                    e = epool.tile([P, 2, 512], BF16, tag="e")
                    nc.scalar.activation(
                        e[:], st[:], mybir.ActivationFunctionType.Exp, scale=0.125
                    )
                    if ck < len(deferred):
                        deferred[ck]()
                        if ck == len(deferred) - 1:
                            deferred = []
                    if qps is not None and ck < 8:
                        nc.tensor.matmul(
                            qps[:],
                            wq[:, ck, pair * P:(pair + 1) * P],
                            xts[s + 1][:, ck, :],
                            start=(ck == 0),
                            stop=(ck == 7),
                        )
                        if ck == 7:
                            nc.vector.tensor_copy(
                                qT[:, pair, (s + 1) * 512:(s + 2) * 512], qps[:])
                    pending.append((e, ck))
                    if len(pending) == 6:
                        emit_pv(pending[:4])
                        pending = pending[4:]
                tail0, tail1 = pending[:2], pending[2:]

                # drain chain pieces: pv copies + denominator reciprocal
                # (DVE), then the PE-side broadcast + multiply.  cell carries
                # rc from the first piece to the second.
                cell = {}

                def copies_rc(pvs=pvs, pair=pair, s=s, cell=cell):
                    # aT[64*sub.., pair, slab] = pv[0:64]; dn row = pv[64]
                    for sub in range(2):
                        nc.vector.tensor_copy(
                            aT[64 * sub:64 * sub + 64, pair,
                               s * 512:(s + 1) * 512],
                            pvs[sub][0:64, :],
                        )
                    dn = npool.tile([1, 2, 512], F32, tag="dn",
                                    name=f"dn_{pair}_{s}")
                    for sub in range(2):
                        nc.vector.tensor_copy(dn[:, sub, :], pvs[sub][64:65, :])
                    rc32 = npool.tile([1, 2, 512], F32, tag="rc32",
                                      name=f"rc32_{pair}_{s}")
                    nc.vector.reciprocal_approx_fast(rc32[:], dn[:])
                    rc = npool.tile([1, 2, 512], BF16, tag="rc",
                                    name=f"rc_{pair}_{s}")
                    nc.vector.tensor_copy(rc[:], rc32[:])
                    cell["rc"] = rc

                def finish_norm(pair=pair, s=s, cell=cell):
                    rc = cell["rc"]
                    bcs = npool.tile([P, 512], BF16, tag="bcs",
                                     name=f"bcs_{pair}_{s}")
                    for sub in range(2):
                        bc = ps_proj.tile([P, 512], F32, tag="proj",
                                          name=f"bc_{pair}_{s}_{sub}")
                        nc.tensor.matmul(bc[0:64, :], ones_row[:], rc[:, sub, :])
                        o = 64 * sub
                        nc.vector.tensor_copy(bcs[o:o + 64, :], bc[0:64, :])
                    sl = aT[:, pair, s * 512:(s + 1) * 512]
                    nc.vector.tensor_mul(sl, sl, bcs[:])

                def make_proj(nck, ct):
                    def f():
                        pp = ps_proj.tile([P, 512], F32, tag="proj",
                                          name=f"proj{nck}_{ct}")
                        for dc in range(4):
                            nc.tensor.matmul(
                                pp[:],
                                aT[:, dc, nck * P:(nck + 1) * P],
                                wp[:, dc, ct * 512:(ct + 1) * 512],
                                start=(dc == 0),
                                stop=(dc == 3),
                            )
                        ot = opool.tile([P, 512], F32, tag="ot")
                        nc.vector.tensor_copy(ot[:], pp[:])
                        nc.sync.dma_start(
                            out_d[nck * P:(nck + 1) * P,
                                  ct * 512:(ct + 1) * 512],
                            ot[:],
                        )
                    return f

                drain = [
                    lambda b=tail0, f=emit_pv: f(b),
                    lambda b=tail1, f=emit_pv: f(b),
                    copies_rc,
                    finish_norm,
                ]
                if pair == 3:
                    # this slab's projection follows its normalization; both
                    # are deferred into the next slab's first chunks (or run
                    # inline for the final slab)
                    projs = [make_proj(4 * s + i, ct)
                             for i in range(4) for ct in range(2)]
                    if s == NSLABS - 1:
                        for f in drain[:2]:
                            f()
                        pwarm = ps_proj.tile([P, 512], F32, tag="proj",
                                             name="proj_warm")
                        for _ in range(32):
                            nc.tensor.matmul(pwarm[:, 0:P], ident[:], ident[:])
                        for f in drain[2:] + projs:
                            f()
                    else:
                        # proj groups wait until chunk 8: by then the next
                        # slab's qT accumulator has released its PSUM ring
                        # slot, so the groups don't contend with it
                        noop = lambda: None
                        deferred = drain + [noop] * 4 + projs
                else:
                    deferred = drain



def shard_inputs(x, W_qkv, W_proj):
    """Full inputs -> 8 per-core in_maps. Core c: batch c//2, head-group c%2."""
    x = np.asarray(x, dtype=np.float32)
    W_qkv = np.asarray(W_qkv, dtype=np.float32)
    W_proj = np.asarray(W_proj, dtype=np.float32)
    in_maps = []
    for core in range(8):
        b, g = core // 2, core % 2
        cols = slice(g * DC, (g + 1) * DC)
        w = np.concatenate(
            [W_qkv[:, 0:C][:, cols], W_qkv[:, C:2 * C][:, cols],
             W_qkv[:, 2 * C:3 * C][:, cols]],
            axis=1,
        )
        in_maps.append({
            "xt": np.ascontiguousarray(x[b].T).astype(ml_dtypes.bfloat16),
            "wqkv": np.ascontiguousarray(w).astype(ml_dtypes.bfloat16),
            "wproj": np.ascontiguousarray(
                W_proj[g * DC:(g + 1) * DC, :]).astype(ml_dtypes.bfloat16),
        })
    return in_maps


def unshard_output(results, b_proj):
    b_proj = np.asarray(b_proj, dtype=np.float32)
    out = np.empty((4, N, C), dtype=np.float32)
    for b in range(4):
        out[b] = results[2 * b]["out"] + results[2 * b + 1]["out"] + b_proj[None, :]
    return out


_NC_CACHE = []


def kernel(x, W_qkv, W_proj, b_proj, trace=False):
    in_maps = shard_inputs(x, W_qkv, W_proj)
    if not _NC_CACHE:
        _NC_CACHE.append(build_program())
    nc = _NC_CACHE[0]
    res = run_bass_kernel_spmd(nc, in_maps, core_ids=list(range(8)), trace=trace)
    out = unshard_output(res.results, b_proj)
    if trace:
        return out, res
    return out


# revision 42
# speedup vs baseline: 1.0177x; 1.0131x over previous
"""Multi-head attention (B=4, N=2048, C=1024, H=16, D=64) on 8 TRN2 NeuronCores.

Sharding: core c handles batch b = c//2 and head-group g = c%2 (8 heads = 512
dims).  Each core computes qkv projection, attention, and a partial output
projection for its head slice; the host sums the two partials per batch and
adds the proj bias.

Per-core kernel (bf16 data, fp32 PSUM accumulation everywhere):
  host prep: x is pre-transposed to [C, N] bf16 per batch (removes all PE
  transposes); W_qkv / W_proj pre-cast to bf16 (halves weight DMA + PE
  weight-load time; bf16 loads hide fully under the 512-row streams).
  phase 1: DMA xT slabs; qT/kT (d-major) and augmented v (n-major, per-head
           65th column of ones) via accumulating matmuls.
  phase 2: per (slab, head-pair): S^T = k @ q^T in PSUM (row-packed pairs)
           -> one exp per chunk on ScalarE (scale=1/8 folded in; no max
           subtraction, logits ~ N(0,1)) -> bf16 e tiles -> PV against
           augmented v gives P@V rows 0..63 + softmax denominator row 64 in
           one accumulation group.  PV trails exp by >=2 chunks and is
           emitted sub-major in 4-chunk blocks so consecutive matmuls
           accumulate into the same PSUM bank (a bank switch costs ~170ns).
           Normalization: one reciprocal over both heads' stacked
           denominators (DVE, emitted at pair end), then the PE-side
           broadcast matmuls + multiply are DEFERRED into the next pair's
           chunk loop so the PE never stalls on the reciprocal chain.
           q^T for slab s+1 is computed inside slab s's pairs (one
           accumulating matmul per chunk over the first 8 chunks) -- the
           attention phase is exp-paced and absorbs it.  The slab's output
           projection (double-buffered PSUM) runs when its last pair
           completes.
  Dummy matmuls at kernel start / phase boundary / tail keep the PE HAM
  clock gate at 2.4 GHz (idle >3.4us re-throttles to 1.2 GHz).
"""

from contextlib import ExitStack

import ml_dtypes
import numpy as np

import concourse.bass as bass
import concourse.tile as tile
from concourse import bacc, mybir
from concourse.bass_utils import run_bass_kernel_spmd

P = 128
N = 2048          # tokens per batch
C = 1024          # model dim
DC = 512          # head dims per core (8 heads x 64)
NSLABS = N // 512
F32 = mybir.dt.float32
BF16 = mybir.dt.bfloat16


def build_program(trace_label: str = "attn"):
    nc = bacc.Bacc("TRN2", target_bir_lowering=False, name=trace_label)
    # x arrives pre-transposed (host-side): [C, N] bf16, so no PE transposes.
    xt_d = nc.dram_tensor("xt", [C, N], BF16, kind="ExternalInput").ap()
    wqkv_d = nc.dram_tensor("wqkv", [C, 3 * DC], BF16, kind="ExternalInput").ap()
    wproj_d = nc.dram_tensor("wproj", [DC, C], BF16, kind="ExternalInput").ap()
    out_d = nc.dram_tensor("out", [N, C], F32, kind="ExternalOutput").ap()

    with tile.TileContext(nc) as tc, ExitStack() as ctx:
        _emit(ctx, tc, xt_d, wqkv_d, wproj_d, out_d)
    nc.compile()
    return nc


def _emit(ctx, tc, xt_d, wqkv_d, wproj_d, out_d):
    nc = tc.nc

    const = ctx.enter_context(tc.tile_pool(name="const", bufs=1))
    # warm-up operand: contents are irrelevant (the HAM clock gate only sees
    # PE activity); a memset is ready in ~100ns vs ~9us for a gpsimd-built
    # identity, so the PE warms while the first DMAs are still in flight
    ident = const.tile([P, P], BF16, tag="ident")
    nc.any.memset(ident.bitcast(mybir.dt.uint32), 0)
    ONE_BF16_BITS = 0x3F803F80  # two packed bf16 1.0s; memset writes raw bits
    ones_row = const.tile([1, 64], BF16, tag="ones_row")  # lhsT for broadcast
    nc.any.memset(ones_row.bitcast(mybir.dt.uint32), ONE_BF16_BITS)

    # Persistent SBUF tensors (d-major q/k in bf16, n-major v in fp8).
    # v is stored augmented: per head 65 columns, the 65th = 1.0, so a single
    # accumulating matmul yields both P@V (rows 0..63) and the softmax
    # denominator (row 64) without a second accumulation group in the bank.
    persist = ctx.enter_context(tc.tile_pool(name="persist", bufs=1))
    qT = persist.tile([P, 4, N], BF16, tag="qT")          # [d%128, d//128, n]
    kT = persist.tile([P, 4, N], BF16, tag="kT")
    va = persist.tile([P, N // P, 8 * 65], BF16, tag="va")  # [n%128, n//128, 65*h+dd]
    nc.any.memset(va[:].bitcast(mybir.dt.uint32), ONE_BF16_BITS)

    # ---------------- phase 1: k/v projection (+ q for slab 0) ----------------
    # wq and the xT tiles stay resident: qT for slab s+1 is computed inside
    # phase 2's slab s (the attention phase is exp-paced, so the PE has slack).
    wpool = ctx.enter_context(tc.tile_pool(name="wqkv", bufs=1))
    xt_pool = ctx.enter_context(tc.tile_pool(name="xt", bufs=4))
    xts = []
    with tc.tile_pool(name="ps_warm", bufs=1, space="PSUM") as ps_warm, \
         tc.tile_pool(name="ps_qkv", bufs=4, space="PSUM") as ps_qkv:

        # PE warm-up: the HAM clock gate needs ~3.4us of sustained matmul
        # activity to lift the PE from 1.2 to 2.4 GHz. Spin harmless matmuls
        # while the first x/W DMAs are in flight so real work starts warm.
        warm = ps_warm.tile([P, P], F32, tag="warm")
        for _ in range(48):
            nc.tensor.matmul(warm[:], ident[:], ident[:])

        # all xT slab DMAs are issued upfront; W rides the scalar-engine DGE
        # queue so it streams in parallel.
        wq = wpool.tile([P, 8, 3 * DC], BF16, tag="wqkv")  # [c%128, c//128, col]
        for s in range(NSLABS):
            xt = xt_pool.tile([P, 8, 512], BF16, tag="xt",
                              name=f"xt{s}")  # [c%128, c//128, n]
            for cc in range(8):
                nc.sync.dma_start(
                    xt[:, cc, :],
                    xt_d[cc * P:(cc + 1) * P, s * 512:(s + 1) * 512],
                )
            xts.append(xt)
            if s == 0:
                for cc in range(8):
                    nc.scalar.dma_start(wq[:, cc, :], wqkv_d[cc * P:(cc + 1) * P, :])

        for s in range(NSLABS):
            xt = xts[s]
            # k^T for every slab; q^T only for slab 0 (rest come in phase 2)
            for dst, base in ((kT, DC), (qT, 0)) if s == 0 else ((kT, DC),):
                for dc in range(4):
                    ps = ps_qkv.tile([P, 512], F32, tag="qkv")
                    col = base + dc * P
                    for cc in range(8):
                        nc.tensor.matmul(
                            ps[:],
                            wq[:, cc, col:col + P],
                            xt[:, cc, :],
                            start=(cc == 0),
                            stop=(cc == 7),
                        )
                    nc.vector.tensor_copy(dst[:, dc, s * 512:(s + 1) * 512], ps[:])
            # v (natural layout, scattered into the 65-wide augmented blocks)
            for i in range(4):
                ps = ps_qkv.tile([P, 512], F32, tag="qkv")
                for cc in range(8):
                    nc.tensor.matmul(
                        ps[:],
                        xt[:, cc, i * P:(i + 1) * P],
                        wq[:, cc, 2 * DC:3 * DC],
                        start=(cc == 0),
                        stop=(cc == 7),
                    )
                for h in range(8):
                    nc.vector.tensor_copy(
                        va[:, 4 * s + i, 65 * h:65 * h + 64],
                        ps[:, 64 * h:64 * h + 64],
                    )

        # keep the PE busy across the phase boundary (PSUM-bank WAR waits
        # would otherwise idle it past the HAM re-throttle window)
        for _ in range(8):
            nc.tensor.matmul(warm[:], ident[:], ident[:])

    # ---------------- phase 2: attention ----------------
    # aT is only written from phase 2 on; allocating it here (after the
    # phase-1 pools release) keeps phase-1 SBUF under budget.
    attn_persist = ctx.enter_context(tc.tile_pool(name="attn_persist", bufs=1))
    aT = attn_persist.tile([P, 4, N], BF16, tag="aT")     # attn_out^T
    with tc.tile_pool(name="ps_st", bufs=2, space="PSUM") as ps_st, \
         tc.tile_pool(name="ps_pv", bufs=2, space="PSUM") as ps_pv, \
         tc.tile_pool(name="ps_proj", bufs=2, space="PSUM") as ps_proj, \
         tc.tile_pool(name="etile", bufs=8) as epool, \
         tc.tile_pool(name="norm", bufs=4) as npool, \
         tc.tile_pool(name="wproj", bufs=1) as wp_pool, \
         tc.tile_pool(name="oproj", bufs=3) as opool:

        wp = wp_pool.tile([P, 4, C], BF16, tag="wp")
        for dc in range(4):
            nc.scalar.dma_start(wp[:, dc, :], wproj_d[dc * P:(dc + 1) * P, :])

        # the previous pair's PV tail, PSUM drain, and normalization are
        # deferred into the next pair's first chunks (one piece per chunk) so
        # the next pair's scores/exp start immediately at the boundary and
        # the exp stream never waits on the drain chain
        deferred = []
        for s in range(NSLABS):          # 512-wide n_q slab (outer: spreads proj)
            for pair in range(4):        # heads (2*pair, 2*pair+1); d-chunk=pair
                pvs = [
                    ps_pv.tile([P, 512], F32, tag="pv", name=f"pv{pair}_{s}_{i}")
                    for i in range(2)
                ]
                def emit_pv(batch, pvs=pvs, pair=pair):
                    # sub-major over a block of chunks: consecutive matmuls
                    # accumulate into the SAME PSUM bank (alternating banks
                    # per matmul costs ~170ns each on HW)
                    for sub in range(2):
                        h = 2 * pair + sub
                        for e_prev, ck_prev in batch:
                            nc.tensor.matmul(
                                pvs[sub][0:65, :],
                                va[:, ck_prev, 65 * h:65 * h + 65],
                                e_prev[:, sub, :],
                                start=(ck_prev == 0),
                                stop=(ck_prev == N // P - 1),
                            )

                # q^T d-chunk `pair` of the NEXT slab rides this pair's
                # exp-paced slack: one accumulating matmul per chunk, spread
                # over the first 8 chunks (the PV pipeline is still filling
                # there, so single-matmul insertions don't starve the exp)
                qps = None
                if s + 1 < NSLABS:
                    qps = ps_proj.tile([P, 512], F32, tag="proj",
                                       name=f"q_{s + 1}_{pair}")

                # software pipeline: PV trails exp by >=2 chunks (so the PE's
                # PV waits are pre-satisfied) and is emitted in blocks of 4
                # chunks for the same-bank runs above
                pending = []
                for ck in range(N // P):  # 128-wide n_k chunk
                    st = ps_st.tile([P, 2, 512], F32, tag="st")
                    for sub in range(2):
                        o = 64 * sub
                        nc.tensor.matmul(
                            st[:, sub, :],
                            kT[o:o + 64, pair, ck * P:(ck + 1) * P],
                            qT[o:o + 64, pair, s * 512:(s + 1) * 512],
                        )
                    e = epool.tile([P, 2, 512], BF16, tag="e")
                    nc.scalar.activation(
                        e[:], st[:], mybir.ActivationFunctionType.Exp, scale=0.125
                    )
                    if ck < len(deferred):
                        deferred[ck]()
                        if ck == len(deferred) - 1:
                            deferred = []
                    if qps is not None and ck < 8:
                        nc.tensor.matmul(
                            qps[:],
                            wq[:, ck, pair * P:(pair + 1) * P],
                            xts[s + 1][:, ck, :],
                            start=(ck == 0),
                            stop=(ck == 7),
                        )
                        if ck == 7:
                            nc.vector.tensor_copy(
                                qT[:, pair, (s + 1) * 512:(s + 2) * 512], qps[:])
                    pending.append((e, ck))
                    if len(pending) == 6:
                        emit_pv(pending[:4])
                        pending = pending[4:]
                tail0, tail1 = pending[:2], pending[2:]

                # drain chain pieces: pv copies + denominator reciprocal
                # (DVE), then the PE-side broadcast + multiply.  cell carries
                # rc from the first piece to the second.
                cell = {}

                def copies_rc(pvs=pvs, pair=pair, s=s, cell=cell):
                    # aT[64*sub.., pair, slab] = pv[0:64]; dn row = pv[64]
                    for sub in range(2):
                        nc.vector.tensor_copy(
                            aT[64 * sub:64 * sub + 64, pair,
                               s * 512:(s + 1) * 512],
                            pvs[sub][0:64, :],
                        )
                    dn = npool.tile([1, 2, 512], F32, tag="dn",
                                    name=f"dn_{pair}_{s}")
                    for sub in range(2):
                        nc.vector.tensor_copy(dn[:, sub, :], pvs[sub][64:65, :])
                    rc32 = npool.tile([1, 2, 512], F32, tag="rc32",
                                      name=f"rc32_{pair}_{s}")
                    nc.vector.reciprocal_approx_fast(rc32[:], dn[:])
                    rc = npool.tile([1, 2, 512], BF16, tag="rc",
                                    name=f"rc_{pair}_{s}")
                    nc.vector.tensor_copy(rc[:], rc32[:])
                    cell["rc"] = rc

                def finish_norm(pair=pair, s=s, cell=cell):
                    rc = cell["rc"]
                    bcs = npool.tile([P, 512], BF16, tag="bcs",
                                     name=f"bcs_{pair}_{s}")
                    for sub in range(2):
                        bc = ps_proj.tile([P, 512], F32, tag="proj",
                                          name=f"bc_{pair}_{s}_{sub}")
                        nc.tensor.matmul(bc[0:64, :], ones_row[:], rc[:, sub, :])
                        o = 64 * sub
                        nc.vector.tensor_copy(bcs[o:o + 64, :], bc[0:64, :])
                    sl = aT[:, pair, s * 512:(s + 1) * 512]
                    nc.vector.tensor_mul(sl, sl, bcs[:])

                def make_proj(nck, ct):
                    def f():
                        pp = ps_proj.tile([P, 512], F32, tag="proj",
                                          name=f"proj{nck}_{ct}")
                        for dc in range(4):
                            nc.tensor.matmul(
                                pp[:],
                                aT[:, dc, nck * P:(nck + 1) * P],
                                wp[:, dc, ct * 512:(ct + 1) * 512],
                                start=(dc == 0),
                                stop=(dc == 3),
                            )
                        ot = opool.tile([P, 512], F32, tag="ot")
                        nc.vector.tensor_copy(ot[:], pp[:])
                        nc.sync.dma_start(
                            out_d[nck * P:(nck + 1) * P,
                                  ct * 512:(ct + 1) * 512],
                            ot[:],
                        )
                    return f

                noop = lambda: None
                drain = [
                    lambda b=tail0, f=emit_pv: f(b),
                    noop,
                    lambda b=tail1, f=emit_pv: f(b),
                    noop,
                    copies_rc,
                    finish_norm,
                ]
                if pair == 3:
                    # this slab's projection follows its normalization; both
                    # are deferred into the next slab's first chunks (or run
                    # inline for the final slab)
                    projs = [make_proj(4 * s + i, ct)
                             for i in range(4) for ct in range(2)]
                    if s == NSLABS - 1:
                        for f in drain[:2]:
                            f()
                        pwarm = ps_proj.tile([P, 512], F32, tag="proj",
                                             name="proj_warm")
                        for _ in range(20):
                            nc.tensor.matmul(pwarm[:, 0:P], ident[:], ident[:])
                        for f in drain[2:] + projs:
                            f()
                    else:
                        # proj groups wait until chunk 8: by then the next
                        # slab's qT accumulator has released its PSUM ring
                        # slot, so the groups don't contend with it
                        deferred = drain + [noop] * 2 + projs
                else:
                    deferred = drain



def shard_inputs(x, W_qkv, W_proj):
    """Full inputs -> 8 per-core in_maps. Core c: batch c//2, head-group c%2."""
    x = np.asarray(x, dtype=np.float32)
    W_qkv = np.asarray(W_qkv, dtype=np.float32)
    W_proj = np.asarray(W_proj, dtype=np.float32)
    in_maps = []
    for core in range(8):
        b, g = core // 2, core % 2
        cols = slice(g * DC, (g + 1) * DC)
        w = np.concatenate(
            [W_qkv[:, 0:C][:, cols], W_qkv[:, C:2 * C][:, cols],
             W_qkv[:, 2 * C:3 * C][:, cols]],
            axis=1,
        )
        in_maps.append({
            "xt": np.ascontiguousarray(x[b].T).astype(ml_dtypes.bfloat16),
            "wqkv": np.ascontiguousarray(w).astype(ml_dtypes.bfloat16),
            "wproj": np.ascontiguousarray(
                W_proj[g * DC:(g + 1) * DC, :]).astype(ml_dtypes.bfloat16),
        })
    return in_maps


def unshard_output(results, b_proj):
    b_proj = np.asarray(b_proj, dtype=np.float32)
    out = np.empty((4, N, C), dtype=np.float32)
    for b in range(4):
        out[b] = results[2 * b]["out"] + results[2 * b + 1]["out"] + b_proj[None, :]
    return out


_NC_CACHE = []


def kernel(x, W_qkv, W_proj, b_proj, trace=False):
    in_maps = shard_inputs(x, W_qkv, W_proj)
    if not _NC_CACHE:
        _NC_CACHE.append(build_program())
    nc = _NC_CACHE[0]
    res = run_bass_kernel_spmd(nc, in_maps, core_ids=list(range(8)), trace=trace)
    out = unshard_output(res.results, b_proj)
    if trace:
        return out, res
    return out
